# revision 4
# baseline (speedup 1.0000x reference)
"""DecoderRNN (LSTM image-caption decoder) on 8 TRN2 NeuronCores.

Sharding: data-parallel over batch B=128 -> 16 rows per core. No collectives.
Host side does the data-dependent index work (length sort, embedding gather)
and packs operands into PE-friendly layouts; the device runs:
  1. X-proj: x_t @ W_ih^T + bias for all 20 steps as one batched matmul sweep
  2. 19 sequential LSTM-cell steps (h @ W_hh^T, gate nonlinearities)
  3. FC to vocab (10000) for all 19*16 rows, fused (psum+bias)*mask epilogue
All matmuls in bf16 with f32 PSUM accumulation.

Perf notes (trace-driven):
  - each LSTM gate group accumulates in its own PSUM bank so the scalar
    engine can start on a gate while later gates are still in matmul
  - the x-projection is injected into each gate's PSUM accumulation via an
    identity matmul, so gate pre-activations are complete in PSUM and the
    scalar engine reads them directly (no vector-engine add pass)
  - bank-clear semantics: only the first matmul of a gate group uses
    start=True (clears the whole bank's has_written bits); every other
    matmul in the group overwrites-where-unset / accumulates-where-set
"""

import numpy as np
import ml_dtypes

B, S, E, H, V = 128, 20, 512, 512, 10000
T = S - 1            # 19 decode steps
NCORES = 8
BC = B // NCORES     # 16 batch rows per core
NSTEP = S            # 20 LSTM cell evaluations (features step + 19 caption steps)
RX = NSTEP * BC      # 320 x-proj rows per core
R = T * BC           # 304 fc rows per core
KC = E // 128        # 4 contraction chunks (E == H == 512)
MT = 4 * H // 128    # 16 gate m-tiles
VT = (V + 127) // 128  # 79 vocab tiles (last one 16 wide)

PROFILE = False      # set True (from test.py) to capture NTFF trace + exec time
LAST_RESULT = None   # BassKernelResults of the last run (for test.py)

_COMPILED = None


def _build():
    import concourse.mybir as mybir
    import concourse.tile as tile
    from concourse import bacc
    from concourse.masks import make_identity
    from contextlib import ExitStack

    f32 = mybir.dt.float32
    bf16 = mybir.dt.bfloat16
    AF = mybir.ActivationFunctionType
    OP = mybir.AluOpType

    nc = bacc.Bacc(None)

    xk = nc.declare_dram_parameter("xk", [KC * 128, RX], bf16, isOutput=False)
    wih = nc.declare_dram_parameter("wih", [KC * 128, 4 * H], bf16, isOutput=False)
    whh = nc.declare_dram_parameter("whh", [KC * 128, 4 * H], bf16, isOutput=False)
    bias = nc.declare_dram_parameter("bias", [128, MT], f32, isOutput=False)
    fcw = nc.declare_dram_parameter("fcw", [KC * 128, V], bf16, isOutput=False)
    fcb = nc.declare_dram_parameter("fcb", [128, VT], f32, isOutput=False)
    mask = nc.declare_dram_parameter("mask", [128, R], f32, isOutput=False)
    preds = nc.declare_dram_parameter("preds", [V, R], f32, isOutput=True)

    xk_r = xk.rearrange("(ko ki) r -> ki ko r", ki=128)
    wih_r = wih.rearrange("(ko ki) m -> ki ko m", ki=128)
    whh_r = whh.rearrange("(ko ki) m -> ki ko m", ki=128)
    fcw_r = fcw.rearrange("(ko ki) m -> ki ko m", ki=128)

    with tile.TileContext(nc) as tc, ExitStack() as ctx:
        const = ctx.enter_context(tc.tile_pool(name="const", bufs=1))
        gates = ctx.enter_context(tc.tile_pool(name="gates", bufs=2))
        fcout = ctx.enter_context(tc.tile_pool(name="fcout", bufs=3))
        # one bank per slot; x-proj + fc share the 4 rotating "big" slots,
        # the four gate groups get a dedicated bank each (4 + 4 <= 8 banks)
        ps_big = ctx.enter_context(tc.tile_pool(name="ps_big", bufs=4, space="PSUM"))
        ps_gate = ctx.enter_context(tc.tile_pool(name="ps_gate", bufs=1, space="PSUM"))

        # ---- stage weights/inputs into SBUF (k-chunk-major layouts) ----
        # ordered so the X-proj dependencies land first
        xk_sb = const.tile([128, KC, RX], bf16)
        wih_sb = const.tile([128, KC, 4 * H], bf16)
        for k in range(KC):
            nc.sync.dma_start(xk_sb[:, k, :], xk_r[:, k, :])
            nc.sync.dma_start(wih_sb[:, k, :], wih_r[:, k, :])
        bias_sb = const.tile([128, MT], f32)
        nc.sync.dma_start(bias_sb[:], bias[:])
        whh_sb = const.tile([128, KC, 4 * H], bf16)
        for k in range(KC):
            nc.sync.dma_start(whh_sb[:, k, :], whh_r[:, k, :])
        mask_sb = const.tile([128, R], f32)
        nc.sync.dma_start(mask_sb[:], mask[:])
        fcb_sb = const.tile([128, VT], f32)
        nc.sync.dma_start(fcb_sb[:], fcb[:])
        fcw_sb = const.tile([128, KC, V], bf16)
        for k in range(KC):
            nc.sync.dma_start(fcw_sb[:, k, :], fcw_r[:, k, :])

        ident = const.tile([128, 128], bf16)
        make_identity(nc, ident[:])

        xp_sb = const.tile([128, MT, RX], bf16)  # x-projections + bias, all steps
        h_all = const.tile([128, KC, RX], bf16)  # h_t for every step (k-major)
        c_sb = const.tile([128, KC, BC], f32)    # cell state

        # ---- X-proj: xp[:, mt, r] = sum_k W_ihT[k, mt*128:+128].T @ x + bias ----
        for mt in range(MT):
            ps = ps_big.tile([128, RX], f32, tag="big")
            for k in range(KC):
                nc.tensor.matmul(
                    ps[:],
                    wih_sb[:, k, mt * 128:(mt + 1) * 128],
                    xk_sb[:, k, :],
                    start=(k == 0),
                    stop=(k == KC - 1),
                )
            nc.scalar.activation(
                xp_sb[:, mt, :], ps[:], AF.Identity,
                bias=bias_sb[:, mt:mt + 1], scale=1.0,
            )

        # gate m-tile bases: torch LSTMCell gate order is i, f, g, o
        I0, F0, G0, O0 = 0, 4, 8, 12

        def xp_t(g0, s):
            return xp_sb[:, g0:g0 + 4, s * BC:(s + 1) * BC]

        # ---- step 0: h,c from features only (h_prev = 0, c_prev = 0) ----
        tg = gates.tile([128, 4, BC], f32)
        nc.scalar.activation(tg[:], xp_t(G0, 0), AF.Tanh)
        si = gates.tile([128, 4, BC], f32)
        nc.scalar.activation(si[:], xp_t(I0, 0), AF.Sigmoid)
        so = gates.tile([128, 4, BC], f32)
        nc.scalar.activation(so[:], xp_t(O0, 0), AF.Sigmoid)
        nc.vector.tensor_tensor(c_sb[:], si[:], tg[:], OP.mult)
        tc_ = gates.tile([128, 4, BC], f32)
        nc.scalar.activation(tc_[:], c_sb[:], AF.Tanh)
        nc.vector.tensor_tensor(h_all[:, :, 0:BC], so[:], tc_[:], OP.mult)

        # ---- steps 1..19: full LSTM cell ----
        # gate group issue order g, f, i, o: the c-update chain (needs g, f, i)
        # runs while the o matmuls stream; the o tail is short
        for s in range(1, NSTEP):
            h_prev = h_all[:, :, (s - 1) * BC:s * BC]

            def gate_mms(g0, tag):
                ps = ps_gate.tile([128, 4, BC], f32, tag=tag)
                first = True
                for j in range(4):
                    for k in range(KC):
                        nc.tensor.matmul(
                            ps[:, j, :],
                            whh_sb[:, k, (g0 + j) * 128:(g0 + j + 1) * 128],
                            h_prev[:, k, :],
                            start=first,
                            stop=False,
                            skip_group_check=True,
                        )
                        first = False
                # inject x-proj (+bias) via identity matmul: completes the
                # gate pre-activation entirely inside PSUM
                nc.tensor.matmul(
                    ps[:, :, :], ident[:], xp_t(g0, s),
                    start=False, stop=True, skip_group_check=True,
                )
                return ps

        # g
            ps_g = gate_mms(G0, "pg_g")
            tg = gates.tile([128, 4, BC], f32)
            nc.scalar.activation(tg[:], ps_g[:, :, :], AF.Tanh)
        # f
            ps_f = gate_mms(F0, "pg_f")
            sf = gates.tile([128, 4, BC], f32)
            nc.scalar.activation(sf[:], ps_f[:, :, :], AF.Sigmoid)
            c1 = gates.tile([128, 4, BC], f32)
            nc.vector.tensor_tensor(c1[:], sf[:], c_sb[:], OP.mult)
        # i
            ps_i = gate_mms(I0, "pg_i")
            si = gates.tile([128, 4, BC], f32)
            nc.scalar.activation(si[:], ps_i[:, :, :], AF.Sigmoid)
            t2 = gates.tile([128, 4, BC], f32)
            nc.vector.tensor_tensor(t2[:], si[:], tg[:], OP.mult)
            nc.vector.tensor_tensor(c_sb[:], c1[:], t2[:], OP.add)
        # o
            ps_o = gate_mms(O0, "pg_o")
            so = gates.tile([128, 4, BC], f32)
            nc.scalar.activation(so[:], ps_o[:, :, :], AF.Sigmoid)
            tc_ = gates.tile([128, 4, BC], f32)
            nc.scalar.activation(tc_[:], c_sb[:], AF.Tanh)
            nc.vector.tensor_tensor(
                h_all[:, :, s * BC:(s + 1) * BC], so[:], tc_[:], OP.mult
            )

        # ---- FC: preds[v, r] = (sum_k fcwT.T @ h + fcb) * mask ----
        h_fc = h_all[:, :, BC:]  # rows for t=0..18 -> [128, KC, R]
        for vt in range(VT):
            vw = 128 if vt < VT - 1 else V - 128 * (VT - 1)
            ps = ps_big.tile([128, R], f32, tag="big")
            for k in range(KC):
                nc.tensor.matmul(
                    ps[:vw, :],
                    fcw_sb[:, k, vt * 128:vt * 128 + vw],
                    h_fc[:, k, :],
                    start=(k == 0),
                    stop=(k == KC - 1),
                )
            # epilogue split across engines: ACT adds the per-vocab bias while
            # draining PSUM, DVE applies the ragged-length mask; out-DMAs
            # alternate between the two DGE sequencers so descriptor
            # generation (~0.6us per 2D transfer) is not a serial bottleneck
            sb1 = fcout.tile([128, R], f32, tag="sb1")
            nc.scalar.activation(
                sb1[:vw, :], ps[:vw, :], AF.Identity,
                bias=fcb_sb[:vw, vt:vt + 1], scale=1.0,
            )
            out_sb = fcout.tile([128, R], f32)
            nc.vector.tensor_tensor(out_sb[:vw, :], sb1[:vw, :], mask_sb[:vw, :],
                                    OP.mult)
            eng = nc.sync if vt % 2 == 0 else nc.gpsimd
            eng.dma_start(preds[vt * 128:vt * 128 + vw, :], out_sb[:vw, :])

    nc.compile()
    return nc


def _get_compiled():
    global _COMPILED
    if _COMPILED is None:
        _COMPILED = _build()
    return _COMPILED


def kernel(images, captions, length, emb, W_ih, W_hh, b_ih, b_hh, fc_w, fc_b):
    global LAST_RESULT
    from concourse.bass_utils import run_bass_kernel_spmd

    images = np.asarray(images)
    captions = np.asarray(captions)
    length = np.asarray(length)
    emb = np.asarray(emb)
    bf = ml_dtypes.bfloat16

    # ---- host: data-dependent index work (tiny) ----
    lens = length[:, 0]
    sort_ind = np.argsort(-lens, kind="stable").astype(np.int32)
    sorted_lens = lens[sort_ind]
    dec_len = (sorted_lens - 1).astype(lens.dtype)
    captions_s = captions[sort_ind]
    features = images[sort_ind].astype(np.float32)          # [B, E]
    embs = np.asarray(emb, np.float32)[captions_s[:, :T]]   # [B, T, E]
    X = np.concatenate([features[:, None, :], embs], axis=1)  # [B, NSTEP, E]

    bias_v = (np.asarray(b_ih, np.float32) + np.asarray(b_hh, np.float32))
    bias_pm = np.ascontiguousarray(bias_v.reshape(MT, 128).T)          # [128, MT]
    wihT = np.ascontiguousarray(np.asarray(W_ih).T).astype(bf)          # [E, 4H]
    whhT = np.ascontiguousarray(np.asarray(W_hh).T).astype(bf)          # [H, 4H]
    fcwT = np.ascontiguousarray(np.asarray(fc_w).T).astype(bf)          # [H, V]
    fcb_pad = np.zeros(VT * 128, np.float32)
    fcb_pad[:V] = np.asarray(fc_b, np.float32)
    fcb_pm = np.ascontiguousarray(fcb_pad.reshape(VT, 128).T)           # [128, VT]

    t_idx = np.arange(T)
    in_maps = []
    for c in range(NCORES):
        rows = slice(c * BC, (c + 1) * BC)
        Xc = X[rows]                                        # [BC, NSTEP, E]
        xk_c = np.ascontiguousarray(
            Xc.transpose(2, 1, 0).reshape(E, RX)).astype(bf)  # [E, NSTEP*BC]
        mask_r = (dec_len[rows][None, :] > t_idx[:, None]).reshape(R)
        mask_full = np.ascontiguousarray(
            np.broadcast_to(mask_r.astype(np.float32), (128, R)))
        in_maps.append(dict(
            xk=xk_c, wih=wihT, whh=whhT, bias=bias_pm,
            fcw=fcwT, fcb=fcb_pm, mask=mask_full,
        ))

    nc = _get_compiled()
    res = run_bass_kernel_spmd(
        nc, in_maps, list(range(NCORES)), trace=PROFILE,
    )
    LAST_RESULT = res

    predictions = np.empty((B, T, V), np.float32)
    for c in range(NCORES):
        pc = res.results[c]["preds"]                        # [V, R]
        predictions[c * BC:(c + 1) * BC] = (
            pc.reshape(V, T, BC).transpose(2, 1, 0))
    return predictions, captions_s, dec_len, sort_ind


# revision 5
# speedup vs baseline: 1.2110x; 1.2110x over previous
"""DecoderRNN (LSTM image-caption decoder) on 8 TRN2 NeuronCores.

Sharding: data-parallel over batch B=128 -> 16 rows per core. No collectives.
Host side does the data-dependent index work (length sort, embedding gather)
and packs operands into PE-friendly layouts; the device runs:
  1. X-proj: x_t @ W_ih^T + bias for all 20 steps as one batched matmul sweep
  2. 19 sequential LSTM-cell steps (h @ W_hh^T, gate nonlinearities)
  3. FC to vocab (10000) for all 19*16 rows, fused (psum+bias)*mask epilogue
All matmuls in bf16 with f32 PSUM accumulation.

Perf notes (trace-driven):
  - each LSTM gate group accumulates in its own PSUM bank so the scalar
    engine can start on a gate while later gates are still in matmul
  - the x-projection is injected into each gate's PSUM accumulation via an
    identity matmul, so gate pre-activations are complete in PSUM and the
    scalar engine reads them directly (no vector-engine add pass)
  - bank-clear semantics: only the first matmul of a gate group uses
    start=True (clears the whole bank's has_written bits); every other
    matmul in the group overwrites-where-unset / accumulates-where-set
"""

import numpy as np
import ml_dtypes

B, S, E, H, V = 128, 20, 512, 512, 10000
T = S - 1            # 19 decode steps
NCORES = 8
BC = B // NCORES     # 16 batch rows per core
NSTEP = S            # 20 LSTM cell evaluations (features step + 19 caption steps)
RX = NSTEP * BC      # 320 x-proj rows per core
R = T * BC           # 304 fc rows per core
KC = E // 128        # 4 contraction chunks (E == H == 512)
MT = 4 * H // 128    # 16 gate m-tiles
VT = (V + 127) // 128  # 79 vocab tiles
VP = VT * 128          # vocab padded to a whole number of tiles

PROFILE = False      # set True (from test.py) to capture NTFF trace + exec time
LAST_RESULT = None   # BassKernelResults of the last run (for test.py)

_COMPILED = None


def _build():
    import concourse.mybir as mybir
    import concourse.tile as tile
    from concourse import bacc
    from concourse.masks import make_identity
    from contextlib import ExitStack

    f32 = mybir.dt.float32
    bf16 = mybir.dt.bfloat16
    AF = mybir.ActivationFunctionType
    OP = mybir.AluOpType

    nc = bacc.Bacc(None)

    xk = nc.declare_dram_parameter("xk", [KC * 128, RX], bf16, isOutput=False)
    wih = nc.declare_dram_parameter("wih", [KC * 128, 4 * H], bf16, isOutput=False)
    whh = nc.declare_dram_parameter("whh", [KC * 128, 4 * H], bf16, isOutput=False)
    bias = nc.declare_dram_parameter("bias", [128, MT], f32, isOutput=False)
    fcw = nc.declare_dram_parameter("fcw", [KC * 128, VP], bf16, isOutput=False)
    fcb = nc.declare_dram_parameter("fcb", [128, VT], f32, isOutput=False)
    mask = nc.declare_dram_parameter("mask", [128, R], f32, isOutput=False)
    preds = nc.declare_dram_parameter("preds", [VP, R], f32, isOutput=True)

    xk_r = xk.rearrange("(ko ki) r -> ki ko r", ki=128)
    wih_r = wih.rearrange("(ko ki) m -> ki ko m", ki=128)
    whh_r = whh.rearrange("(ko ki) m -> ki ko m", ki=128)
    fcw_r = fcw.rearrange("(ko ki) m -> ki ko m", ki=128)

    with tile.TileContext(nc) as tc, ExitStack() as ctx:
        const = ctx.enter_context(tc.tile_pool(name="const", bufs=1))
        gates = ctx.enter_context(tc.tile_pool(name="gates", bufs=2))
        fcout = ctx.enter_context(tc.tile_pool(name="fcout", bufs=3))
        # one bank per slot; x-proj + fc share the 4 rotating "big" slots,
        # the four gate groups get a dedicated bank each (4 + 4 <= 8 banks)
        ps_big = ctx.enter_context(tc.tile_pool(name="ps_big", bufs=4, space="PSUM"))
        ps_gate = ctx.enter_context(tc.tile_pool(name="ps_gate", bufs=1, space="PSUM"))

        # ---- stage weights/inputs into SBUF (k-chunk-major layouts) ----
        # ordered so the X-proj dependencies land first
        xk_sb = const.tile([128, KC, RX], bf16)
        wih_sb = const.tile([128, KC, 4 * H], bf16)
        for k in range(KC):
            nc.sync.dma_start(xk_sb[:, k, :], xk_r[:, k, :])
            for mq in range(4):
                nc.sync.dma_start(wih_sb[:, k, mq * 512:(mq + 1) * 512],
                                  wih_r[:, k, mq * 512:(mq + 1) * 512])
        bias_sb = const.tile([128, MT], f32)
        nc.sync.dma_start(bias_sb[:], bias[:])
        whh_sb = const.tile([128, KC, 4 * H], bf16)
        for k in range(KC):
            nc.sync.dma_start(whh_sb[:, k, :], whh_r[:, k, :])
        mask_sb = const.tile([128, R], f32)
        nc.sync.dma_start(mask_sb[:], mask[:])
        fcb_sb = const.tile([128, VT], f32)
        nc.sync.dma_start(fcb_sb[:], fcb[:])
        fcw_sb = const.tile([128, KC, VP], bf16)
        for k in range(KC):
            nc.sync.dma_start(fcw_sb[:, k, :], fcw_r[:, k, :])

        ident = const.tile([128, 128], bf16)
        make_identity(nc, ident[:])

        xp_sb = const.tile([128, MT, RX], bf16)  # x-projections + bias, all steps
        h_all = const.tile([128, KC, RX], bf16)  # h_t for every step (k-major)
        c_sb = const.tile([128, KC, BC], f32)    # cell state

        # ---- X-proj: xp[:, mt, r] = sum_k W_ihT[k, mt*128:+128].T @ x + bias ----
        for mt in range(MT):
            ps = ps_big.tile([128, RX], f32, tag="big")
            for k in range(KC):
                nc.tensor.matmul(
                    ps[:],
                    wih_sb[:, k, mt * 128:(mt + 1) * 128],
                    xk_sb[:, k, :],
                    start=(k == 0),
                    stop=(k == KC - 1),
                )
            nc.scalar.activation(
                xp_sb[:, mt, :], ps[:], AF.Identity,
                bias=bias_sb[:, mt:mt + 1], scale=1.0,
            )

        # gate m-tile bases: torch LSTMCell gate order is i, f, g, o
        I0, F0, G0, O0 = 0, 4, 8, 12

        def xp_t(g0, s):
            return xp_sb[:, g0:g0 + 4, s * BC:(s + 1) * BC]

        # ---- step 0: h,c from features only (h_prev = 0, c_prev = 0) ----
        tg = gates.tile([128, 4, BC], f32)
        nc.scalar.activation(tg[:], xp_t(G0, 0), AF.Tanh)
        si = gates.tile([128, 4, BC], f32)
        nc.scalar.activation(si[:], xp_t(I0, 0), AF.Sigmoid)
        so = gates.tile([128, 4, BC], f32)
        nc.scalar.activation(so[:], xp_t(O0, 0), AF.Sigmoid)
        nc.vector.tensor_tensor(c_sb[:], si[:], tg[:], OP.mult)
        tc_ = gates.tile([128, 4, BC], f32)
        nc.scalar.activation(tc_[:], c_sb[:], AF.Tanh)
        nc.vector.tensor_tensor(h_all[:, :, 0:BC], so[:], tc_[:], OP.mult)

        # ---- steps 1..19: full LSTM cell ----
        # gate group issue order g, f, i, o: the c-update chain (needs g, f, i)
        # runs while the o matmuls stream; the o tail is short
        for s in range(1, NSTEP):
            h_prev = h_all[:, :, (s - 1) * BC:s * BC]

            def gate_mms(g0, tag):
                ps = ps_gate.tile([128, 4, BC], f32, tag=tag)
                first = True
                for j in range(4):
                    for k in range(KC):
                        nc.tensor.matmul(
                            ps[:, j, :],
                            whh_sb[:, k, (g0 + j) * 128:(g0 + j + 1) * 128],
                            h_prev[:, k, :],
                            start=first,
                            stop=False,
                            skip_group_check=True,
                        )
                        first = False
                # inject x-proj (+bias) via identity matmul: completes the
                # gate pre-activation entirely inside PSUM
                nc.tensor.matmul(
                    ps[:, :, :], ident[:], xp_t(g0, s),
                    start=False, stop=True, skip_group_check=True,
                )
                return ps

        # g
            ps_g = gate_mms(G0, "pg_g")
            tg = gates.tile([128, 4, BC], f32)
            nc.scalar.activation(tg[:], ps_g[:, :, :], AF.Tanh)
        # f
            ps_f = gate_mms(F0, "pg_f")
            sf = gates.tile([128, 4, BC], f32)
            nc.scalar.activation(sf[:], ps_f[:, :, :], AF.Sigmoid)
            c1 = gates.tile([128, 4, BC], f32)
            nc.vector.tensor_tensor(c1[:], sf[:], c_sb[:], OP.mult)
        # i
            ps_i = gate_mms(I0, "pg_i")
            si = gates.tile([128, 4, BC], f32)
            nc.scalar.activation(si[:], ps_i[:, :, :], AF.Sigmoid)
            t2 = gates.tile([128, 4, BC], f32)
            nc.vector.tensor_tensor(t2[:], si[:], tg[:], OP.mult)
            nc.vector.tensor_tensor(c_sb[:], c1[:], t2[:], OP.add)
        # o
            ps_o = gate_mms(O0, "pg_o")
            so = gates.tile([128, 4, BC], f32)
            nc.scalar.activation(so[:], ps_o[:, :, :], AF.Sigmoid)
            tc_ = gates.tile([128, 4, BC], f32)
            nc.scalar.activation(tc_[:], c_sb[:], AF.Tanh)
            nc.vector.tensor_tensor(
                h_all[:, :, s * BC:(s + 1) * BC], so[:], tc_[:], OP.mult
            )

        # ---- FC: preds[v, r] = (sum_k fcwT.T @ h + fcb) * mask ----
        # vocab padded to 79 uniform 128-wide tiles; the fused
        # (psum + bias) * mask epilogue lands in a 4-tile staging buffer so
        # one 2D DMA covers four vocab tiles (descriptor gen is ~0.6us each)
        h_fc = h_all[:, :, BC:]  # rows for t=0..18 -> [128, KC, R]
        for g0 in range(0, VT, 4):
            gn = min(4, VT - g0)
            stage = fcout.tile([128, 4, R], f32, tag="fcstage")
            for gj in range(gn):
                vt = g0 + gj
                ps = ps_big.tile([128, R], f32, tag="big")
                for k in range(KC):
                    nc.tensor.matmul(
                        ps[:, :],
                        fcw_sb[:, k, vt * 128:(vt + 1) * 128],
                        h_fc[:, k, :],
                        start=(k == 0),
                        stop=(k == KC - 1),
                    )
                nc.vector.scalar_tensor_tensor(
                    stage[:, gj, :], ps[:, :], fcb_sb[:, vt:vt + 1], mask_sb[:, :],
                    OP.add, OP.mult,
                )
            eng = nc.sync if (g0 // 4) % 2 == 0 else nc.gpsimd
            eng.dma_start(
                preds.rearrange("(vt p) r -> p vt r", p=128)[:, g0:g0 + gn, :],
                stage[:, :gn, :],
            )

    nc.compile()
    return nc


def _get_compiled():
    global _COMPILED
    if _COMPILED is None:
        _COMPILED = _build()
    return _COMPILED


def kernel(images, captions, length, emb, W_ih, W_hh, b_ih, b_hh, fc_w, fc_b):
    global LAST_RESULT
    from concourse.bass_utils import run_bass_kernel_spmd

    images = np.asarray(images)
    captions = np.asarray(captions)
    length = np.asarray(length)
    emb = np.asarray(emb)
    bf = ml_dtypes.bfloat16

    # ---- host: data-dependent index work (tiny) ----
    lens = length[:, 0]
    sort_ind = np.argsort(-lens, kind="stable").astype(np.int32)
    sorted_lens = lens[sort_ind]
    dec_len = (sorted_lens - 1).astype(lens.dtype)
    captions_s = captions[sort_ind]
    features = images[sort_ind].astype(np.float32)          # [B, E]
    embs = np.asarray(emb, np.float32)[captions_s[:, :T]]   # [B, T, E]
    X = np.concatenate([features[:, None, :], embs], axis=1)  # [B, NSTEP, E]

    bias_v = (np.asarray(b_ih, np.float32) + np.asarray(b_hh, np.float32))
    bias_pm = np.ascontiguousarray(bias_v.reshape(MT, 128).T)          # [128, MT]
    wihT = np.ascontiguousarray(np.asarray(W_ih).T).astype(bf)          # [E, 4H]
    whhT = np.ascontiguousarray(np.asarray(W_hh).T).astype(bf)          # [H, 4H]
    fcwT = np.zeros((E, VP), bf)                                        # [H, Vpad]
    fcwT[:, :V] = np.asarray(fc_w).T.astype(bf)
    fcb_pad = np.zeros(VT * 128, np.float32)
    fcb_pad[:V] = np.asarray(fc_b, np.float32)
    fcb_pm = np.ascontiguousarray(fcb_pad.reshape(VT, 128).T)           # [128, VT]

    t_idx = np.arange(T)
    in_maps = []
    for c in range(NCORES):
        rows = slice(c * BC, (c + 1) * BC)
        Xc = X[rows]                                        # [BC, NSTEP, E]
        xk_c = np.ascontiguousarray(
            Xc.transpose(2, 1, 0).reshape(E, RX)).astype(bf)  # [E, NSTEP*BC]
        mask_r = (dec_len[rows][None, :] > t_idx[:, None]).reshape(R)
        mask_full = np.ascontiguousarray(
            np.broadcast_to(mask_r.astype(np.float32), (128, R)))
        in_maps.append(dict(
            xk=xk_c, wih=wihT, whh=whhT, bias=bias_pm,
            fcw=fcwT, fcb=fcb_pm, mask=mask_full,
        ))

    nc = _get_compiled()
    res = run_bass_kernel_spmd(
        nc, in_maps, list(range(NCORES)), trace=PROFILE,
    )
    LAST_RESULT = res

    predictions = np.empty((B, T, V), np.float32)
    for c in range(NCORES):
        pc = res.results[c]["preds"][:V]                    # [V, R]
        predictions[c * BC:(c + 1) * BC] = (
            pc.reshape(V, T, BC).transpose(2, 1, 0))
    return predictions, captions_s, dec_len, sort_ind


# revision 7
# speedup vs baseline: 1.2529x; 1.0346x over previous
"""DecoderRNN (LSTM image-caption decoder) on 8 TRN2 NeuronCores.

Sharding: data-parallel over batch B=128 -> 16 rows per core. No collectives.
Host side does the data-dependent index work (length sort, embedding gather)
and packs operands into PE-friendly layouts; the device runs:
  1. X-proj: x_t @ W_ih^T + bias for all 20 steps as one batched matmul sweep
  2. 19 sequential LSTM-cell steps (h @ W_hh^T, gate nonlinearities)
  3. FC to vocab (10000) for all 19*16 rows, fused (psum+bias)*mask epilogue
All matmuls in bf16 with f32 PSUM accumulation.

Perf notes (trace-driven):
  - each LSTM gate group accumulates in its own PSUM bank so the scalar
    engine can start on a gate while later gates are still in matmul
  - the x-projection is injected into each gate's PSUM accumulation via an
    identity matmul, so gate pre-activations are complete in PSUM and the
    scalar engine reads them directly (no vector-engine add pass)
  - bank-clear semantics: only the first matmul of a gate group uses
    start=True (clears the whole bank's has_written bits); every other
    matmul in the group overwrites-where-unset / accumulates-where-set
"""

import numpy as np
import ml_dtypes

B, S, E, H, V = 128, 20, 512, 512, 10000
T = S - 1            # 19 decode steps
NCORES = 8
BC = B // NCORES     # 16 batch rows per core
NSTEP = S            # 20 LSTM cell evaluations (features step + 19 caption steps)
RX = NSTEP * BC      # 320 x-proj rows per core
R = T * BC           # 304 fc rows per core
KC = E // 128        # 4 contraction chunks (E == H == 512)
MT = 4 * H // 128    # 16 gate m-tiles
VT = (V + 127) // 128  # 79 vocab tiles
VP = VT * 128          # vocab padded to a whole number of tiles

PROFILE = False      # set True (from test.py) to capture NTFF trace + exec time
LAST_RESULT = None   # BassKernelResults of the last run (for test.py)

_COMPILED = None


def _build():
    import concourse.mybir as mybir
    import concourse.tile as tile
    from concourse import bacc
    from concourse.masks import make_identity
    from contextlib import ExitStack

    f32 = mybir.dt.float32
    bf16 = mybir.dt.bfloat16
    AF = mybir.ActivationFunctionType
    OP = mybir.AluOpType

    nc = bacc.Bacc(None)

    xk = nc.declare_dram_parameter("xk", [KC * 128, RX], bf16, isOutput=False)
    wih = nc.declare_dram_parameter("wih", [KC * 128, 4 * H], bf16, isOutput=False)
    whh = nc.declare_dram_parameter("whh", [KC * 128, 4 * H], bf16, isOutput=False)
    bias = nc.declare_dram_parameter("bias", [128, MT], f32, isOutput=False)
    fcw = nc.declare_dram_parameter("fcw", [KC * 128, VP], bf16, isOutput=False)
    fcb = nc.declare_dram_parameter("fcb", [128, VT], f32, isOutput=False)
    mask = nc.declare_dram_parameter("mask", [128, R], f32, isOutput=False)
    preds = nc.declare_dram_parameter("preds", [VP, R], bf16, isOutput=True)

    xk_r = xk.rearrange("(ko ki) r -> ki ko r", ki=128)
    wih_r = wih.rearrange("(ko ki) m -> ki ko m", ki=128)
    whh_r = whh.rearrange("(ko ki) m -> ki ko m", ki=128)
    fcw_r = fcw.rearrange("(ko ki) m -> ki ko m", ki=128)

    with tile.TileContext(nc) as tc, ExitStack() as ctx:
        const = ctx.enter_context(tc.tile_pool(name="const", bufs=1))
        gates = ctx.enter_context(tc.tile_pool(name="gates", bufs=2))
        fcout = ctx.enter_context(tc.tile_pool(name="fcout", bufs=3))
        # one bank per slot; x-proj + fc share the 4 rotating "big" slots,
        # the four gate groups get a dedicated bank each (4 + 4 <= 8 banks)
        ps_big = ctx.enter_context(tc.tile_pool(name="ps_big", bufs=4, space="PSUM"))
        ps_gate = ctx.enter_context(tc.tile_pool(name="ps_gate", bufs=1, space="PSUM"))

        # ---- stage weights/inputs into SBUF (k-chunk-major layouts) ----
        # ordered so the X-proj dependencies land first
        xk_sb = const.tile([128, KC, RX], bf16)
        wih_sb = const.tile([128, KC, 4 * H], bf16)
        for k in range(KC):
            nc.sync.dma_start(xk_sb[:, k, :], xk_r[:, k, :])
            for mq in range(4):
                nc.sync.dma_start(wih_sb[:, k, mq * 512:(mq + 1) * 512],
                                  wih_r[:, k, mq * 512:(mq + 1) * 512])
        bias_sb = const.tile([128, MT], f32)
        nc.sync.dma_start(bias_sb[:], bias[:])
        whh_sb = const.tile([128, KC, 4 * H], bf16)
        for k in range(KC):
            nc.sync.dma_start(whh_sb[:, k, :], whh_r[:, k, :])
        mask_sb = const.tile([128, R], f32)
        nc.sync.dma_start(mask_sb[:], mask[:])
        fcb_sb = const.tile([128, VT], f32)
        nc.sync.dma_start(fcb_sb[:], fcb[:])
        fcw_sb = const.tile([128, KC, VP], bf16)
        for k in range(KC):
            nc.sync.dma_start(fcw_sb[:, k, :], fcw_r[:, k, :])

        ident = const.tile([128, 128], bf16)
        make_identity(nc, ident[:])

        # PE warmup: ~4us of dummy matmuls while the input DMAs stream, so the
        # HAM clock gate reaches 2.4 GHz before the first real matmul
        ps_warm = ps_gate.tile([128, 4, BC], f32, tag="pg_g")
        for _ in range(40):
            nc.tensor.matmul(ps_warm[:, :, :], ident[:],
                             ident[:, :4 * BC].rearrange("p (a b) -> p a b", b=BC),
                             start=True, stop=True, skip_group_check=True)

        xp_sb = const.tile([128, MT, RX], bf16)  # x-projections + bias, all steps
        h_all = const.tile([128, KC, RX], bf16)  # h_t for every step (k-major)
        c_sb = const.tile([128, KC, BC], f32)    # cell state

        # ---- X-proj: xp[:, mt, r] = sum_k W_ihT[k, mt*128:+128].T @ x + bias ----
        for mt in range(MT):
            ps = ps_big.tile([128, RX], f32, tag="big")
            for k in range(KC):
                nc.tensor.matmul(
                    ps[:],
                    wih_sb[:, k, mt * 128:(mt + 1) * 128],
                    xk_sb[:, k, :],
                    start=(k == 0),
                    stop=(k == KC - 1),
                )
            nc.scalar.activation(
                xp_sb[:, mt, :], ps[:], AF.Identity,
                bias=bias_sb[:, mt:mt + 1], scale=1.0,
            )

        # gate m-tile bases: torch LSTMCell gate order is i, f, g, o
        I0, F0, G0, O0 = 0, 4, 8, 12

        def xp_t(g0, s):
            return xp_sb[:, g0:g0 + 4, s * BC:(s + 1) * BC]

        # ---- step 0: h,c from features only (h_prev = 0, c_prev = 0) ----
        tg = gates.tile([128, 4, BC], f32)
        nc.scalar.activation(tg[:], xp_t(G0, 0), AF.Tanh)
        si = gates.tile([128, 4, BC], f32)
        nc.scalar.activation(si[:], xp_t(I0, 0), AF.Sigmoid)
        so = gates.tile([128, 4, BC], f32)
        nc.scalar.activation(so[:], xp_t(O0, 0), AF.Sigmoid)
        nc.vector.tensor_tensor(c_sb[:], si[:], tg[:], OP.mult)
        tc_ = gates.tile([128, 4, BC], f32)
        nc.scalar.activation(tc_[:], c_sb[:], AF.Tanh)
        nc.vector.tensor_tensor(h_all[:, :, 0:BC], so[:], tc_[:], OP.mult)

        # ---- steps 1..19: full LSTM cell ----
        # gate group issue order g, f, i, o: the c-update chain (needs g, f, i)
        # runs while the o matmuls stream; the o tail is short
        for s in range(1, NSTEP):
            h_prev = h_all[:, :, (s - 1) * BC:s * BC]

            def gate_mms(g0, tag):
                ps = ps_gate.tile([128, 4, BC], f32, tag=tag)
                first = True
                for j in range(4):
                    for k in range(KC):
                        nc.tensor.matmul(
                            ps[:, j, :],
                            whh_sb[:, k, (g0 + j) * 128:(g0 + j + 1) * 128],
                            h_prev[:, k, :],
                            start=first,
                            stop=False,
                            skip_group_check=True,
                        )
                        first = False
                # inject x-proj (+bias) via identity matmul: completes the
                # gate pre-activation entirely inside PSUM
                nc.tensor.matmul(
                    ps[:, :, :], ident[:], xp_t(g0, s),
                    start=False, stop=True, skip_group_check=True,
                )
                return ps

        # g
            ps_g = gate_mms(G0, "pg_g")
            tg = gates.tile([128, 4, BC], f32)
            nc.scalar.activation(tg[:], ps_g[:, :, :], AF.Tanh)
        # f
            ps_f = gate_mms(F0, "pg_f")
            sf = gates.tile([128, 4, BC], f32)
            nc.scalar.activation(sf[:], ps_f[:, :, :], AF.Sigmoid)
            c1 = gates.tile([128, 4, BC], f32)
            nc.vector.tensor_tensor(c1[:], sf[:], c_sb[:], OP.mult)
        # i
            ps_i = gate_mms(I0, "pg_i")
            si = gates.tile([128, 4, BC], f32)
            nc.scalar.activation(si[:], ps_i[:, :, :], AF.Sigmoid)
            t2 = gates.tile([128, 4, BC], f32)
            nc.vector.tensor_tensor(t2[:], si[:], tg[:], OP.mult)
            nc.vector.tensor_tensor(c_sb[:], c1[:], t2[:], OP.add)
        # o
            ps_o = gate_mms(O0, "pg_o")
            so = gates.tile([128, 4, BC], f32)
            nc.scalar.activation(so[:], ps_o[:, :, :], AF.Sigmoid)
            tc_ = gates.tile([128, 4, BC], f32)
            nc.scalar.activation(tc_[:], c_sb[:], AF.Tanh)
            nc.vector.tensor_tensor(
                h_all[:, :, s * BC:(s + 1) * BC], so[:], tc_[:], OP.mult
            )

        # ---- FC: preds[v, r] = (sum_k fcwT.T @ h + fcb) * mask ----
        # vocab padded to 79 uniform 128-wide tiles; the fused
        # (psum + bias) * mask epilogue lands in a 4-tile staging buffer so
        # one 2D DMA covers four vocab tiles (descriptor gen is ~0.6us each)
        h_fc = h_all[:, :, BC:]  # rows for t=0..18 -> [128, KC, R]
        for g0 in range(0, VT, 4):
            gn = min(4, VT - g0)
            stage = fcout.tile([128, 4, R], bf16, tag="fcstage")
            for gj in range(gn):
                vt = g0 + gj
                ps = ps_big.tile([128, R], f32, tag="big")
                for k in range(KC):
                    nc.tensor.matmul(
                        ps[:, :],
                        fcw_sb[:, k, vt * 128:(vt + 1) * 128],
                        h_fc[:, k, :],
                        start=(k == 0),
                        stop=(k == KC - 1),
                    )
                nc.vector.scalar_tensor_tensor(
                    stage[:, gj, :], ps[:, :], fcb_sb[:, vt:vt + 1], mask_sb[:, :],
                    OP.add, OP.mult,
                )
            eng = nc.sync if (g0 // 4) % 2 == 0 else nc.gpsimd
            eng.dma_start(
                preds.rearrange("(vt p) r -> p vt r", p=128)[:, g0:g0 + gn, :],
                stage[:, :gn, :],
            )

    nc.compile()
    return nc


def _get_compiled():
    global _COMPILED
    if _COMPILED is None:
        _COMPILED = _build()
    return _COMPILED


def kernel(images, captions, length, emb, W_ih, W_hh, b_ih, b_hh, fc_w, fc_b):
    global LAST_RESULT
    from concourse.bass_utils import run_bass_kernel_spmd

    images = np.asarray(images)
    captions = np.asarray(captions)
    length = np.asarray(length)
    emb = np.asarray(emb)
    bf = ml_dtypes.bfloat16

    # ---- host: data-dependent index work (tiny) ----
    lens = length[:, 0]
    sort_ind = np.argsort(-lens, kind="stable").astype(np.int32)
    sorted_lens = lens[sort_ind]
    dec_len = (sorted_lens - 1).astype(lens.dtype)
    captions_s = captions[sort_ind]
    features = images[sort_ind].astype(np.float32)          # [B, E]
    embs = np.asarray(emb, np.float32)[captions_s[:, :T]]   # [B, T, E]
    X = np.concatenate([features[:, None, :], embs], axis=1)  # [B, NSTEP, E]

    bias_v = (np.asarray(b_ih, np.float32) + np.asarray(b_hh, np.float32))
    bias_pm = np.ascontiguousarray(bias_v.reshape(MT, 128).T)          # [128, MT]
    wihT = np.ascontiguousarray(np.asarray(W_ih).T).astype(bf)          # [E, 4H]
    whhT = np.ascontiguousarray(np.asarray(W_hh).T).astype(bf)          # [H, 4H]
    fcwT = np.zeros((E, VP), bf)                                        # [H, Vpad]
    fcwT[:, :V] = np.asarray(fc_w).T.astype(bf)
    fcb_pad = np.zeros(VT * 128, np.float32)
    fcb_pad[:V] = np.asarray(fc_b, np.float32)
    fcb_pm = np.ascontiguousarray(fcb_pad.reshape(VT, 128).T)           # [128, VT]

    t_idx = np.arange(T)
    in_maps = []
    for c in range(NCORES):
        rows = slice(c * BC, (c + 1) * BC)
        Xc = X[rows]                                        # [BC, NSTEP, E]
        xk_c = np.ascontiguousarray(
            Xc.transpose(2, 1, 0).reshape(E, RX)).astype(bf)  # [E, NSTEP*BC]
        mask_r = (dec_len[rows][None, :] > t_idx[:, None]).reshape(R)
        mask_full = np.ascontiguousarray(
            np.broadcast_to(mask_r.astype(np.float32), (128, R)))
        in_maps.append(dict(
            xk=xk_c, wih=wihT, whh=whhT, bias=bias_pm,
            fcw=fcwT, fcb=fcb_pm, mask=mask_full,
        ))

    nc = _get_compiled()
    res = run_bass_kernel_spmd(
        nc, in_maps, list(range(NCORES)), trace=PROFILE,
    )
    LAST_RESULT = res

    predictions = np.empty((B, T, V), np.float32)
    for c in range(NCORES):
        pc = np.asarray(res.results[c]["preds"][:V], np.float32)  # [V, R]
        predictions[c * BC:(c + 1) * BC] = (
            pc.reshape(V, T, BC).transpose(2, 1, 0))
    return predictions, captions_s, dec_len, sort_ind
